# revision 23
# baseline (speedup 1.0000x reference)
"""Trainium2 Bass kernel for batched EEG masking-preserve-order (ragged gather).

Contract: kernel(x, noise, lengths) takes FULL inputs (N=64, L=512, D=840),
shards the batch over 8 NeuronCores (8 samples each), runs a Bass/Tile
kernel per core, and reassembles full-shape outputs:
  (masked_x [N,L,D], masked_attention_mask [N,L],
   masked_attention_mask_invert [N,L], removed_mask [N,L])

Per-sample algorithm (validated bit-exact vs the jax reference):
  clipl  = max(lengths, 16);  len_keep = floor(clipl/2)
  cand_i = i < clipl-1
  veff_i = cand_i ? noise_i : 9.0
  rank_i = #{j : veff_j < veff_i}        (no ties among candidates)
  keep_i = cand_i & (rank_i < len_keep)
  c_i    = inclusive prefix sum of keep  (PE matmuls w/ triangular ones)
  src_j  = #{i : c_i <= j}               (position of (j+1)-th kept row;
                                          = 512 OOB when j >= len_keep)
  out[j] = x[src_j] for j < len_keep, else 0.
Heavy data movement: per (sample, 128-row output chunk) one indirect-DMA
gather with [128,1] row offsets + one indirect-DMA scatter-store (both skip
masked rows via bounds_check OOB). Unwritten output rows rely on pre-zeroed
ExternalOutput buffers. veff row replication uses a DRAM-scratch broadcast
read; rank counting is split between the Vector (is_lt count) and Scalar
(Sign-sum) engines; PE matmuls use bf16 0/1 operands (exact, full-rate).
"""

import numpy as np

N, L, D = 64, 512, 840
NCORES = 8
NS = N // NCORES        # samples per core
P = 128
NCH = L // P            # position chunks per sample
JCH = 2                 # output-row chunks per sample (len_keep <= 256)
G = 2                   # samples per prefix-matmul batch
NGRP = NS // G

_nc_cache = {}


def build_nc():
    import concourse.bass as bass
    import concourse.bacc as bacc
    import concourse.mybir as mybir
    from concourse.tile import TileContext

    f32 = mybir.dt.float32
    bf16 = mybir.dt.bfloat16
    i32 = mybir.dt.int32
    Alu = mybir.AluOpType
    Act = mybir.ActivationFunctionType

    nc = bacc.Bacc()

    x_d = nc.dram_tensor("x", [NS * (L + 1), D], f32, kind="ExternalInput")
    noise_d = nc.dram_tensor("noise", [NS, L], f32, kind="ExternalInput")
    len_d = nc.dram_tensor("lengths", [NS], i32, kind="ExternalInput")
    CMB = 2 * P + NCH + 1 + JCH * P
    cmb_d = nc.dram_tensor("cmb", [P, CMB], f32, kind="ExternalInput")
    iotar_d = nc.dram_tensor("iotar", [NS, L], f32, kind="ExternalInput")

    ox_d = nc.dram_tensor("out_x", [NS * L, D], f32, kind="ExternalOutput")
    om_d = nc.dram_tensor("out_m", [NS, L], f32, kind="ExternalOutput")
    omi_d = nc.dram_tensor("out_mi", [NS, L], f32, kind="ExternalOutput")
    orm_d = nc.dram_tensor("out_r", [NS, L], f32, kind="ExternalOutput")
    vscr_d = nc.dram_tensor("veff_scr", [NS, L], f32)

    with TileContext(nc) as tc:
        with (
            tc.tile_pool(name="const", bufs=1) as cp,
            tc.tile_pool(name="work", bufs=4) as wp,
            tc.tile_pool(name="gp", bufs=4) as gp,
            tc.tile_pool(name="psb", bufs=2, space="PSUM") as psb,
            tc.tile_pool(name="pss", bufs=2, space="PSUM") as pss,
        ):
            # ---------- setup ----------
            # data-path loads first (they gate the whole pipeline)
            noise_t = cp.tile([NS, L], f32, tag="noise")
            nc.sync.dma_start(out=noise_t[:, :], in_=noise_d[:, :])
            lenc_i = cp.tile([NS, 1], i32, tag="lenc_i")
            nc.sync.dma_start(out=lenc_i[:, :], in_=len_d[:, None])
            lenr_i = cp.tile([1, NS], i32, tag="lenr_i")
            nc.sync.dma_start(out=lenr_i[:, :], in_=len_d[None, :])
            iotar_t = cp.tile([NS, L], f32, tag="iotar")
            nc.sync.dma_start(out=iotar_t[:, :], in_=iotar_d[:, :])
            # all [128,*] constants in one combined load
            cmb_t = cp.tile([P, CMB], f32, tag="cmb")
            nc.sync.dma_start(out=cmb_t[:, :], in_=cmb_d[:, :])
            ident_t = cmb_t[:, 0:P]
            triu_t = cmb_t[:, P:2 * P]
            iota4_t = cmb_t[:, 2 * P:2 * P + NCH]
            iotae_t = cmb_t[:, 2 * P + NCH:2 * P + NCH + 1]
            jrow_t = cmb_t[:, 2 * P + NCH + 1:CMB]

            ones1 = cp.tile([1, P], f32, tag="ones1")
            nc.vector.memset(ones1[:, :], 1.0)
            ones128 = cp.tile([P, P], f32, tag="ones128")
            nc.vector.memset(ones128[:, :], 1.0)
            triu_bf = cp.tile([P, P], bf16, tag="triu_bf")
            nc.vector.tensor_copy(out=triu_bf[:, :], in_=triu_t[:, :])
            ones128_bf = cp.tile([P, P], bf16, tag="ones128_bf")
            nc.vector.tensor_copy(out=ones128_bf[:, :], in_=ones128[:, :])

            # column (per-sample-on-partition) scalars
            lenc_f = cp.tile([NS, 1], f32, tag="lenc_f")
            nc.vector.tensor_copy(out=lenc_f[:, :], in_=lenc_i[:, :])
            clipl_c = cp.tile([NS, 1], f32, tag="clipl_c")
            nc.vector.tensor_scalar(
                out=clipl_c[:, :], in0=lenc_f[:, :],
                scalar1=16.0, scalar2=None, op0=Alu.max)
            thr_c = cp.tile([NS, 1], f32, tag="thr_c")
            nc.vector.tensor_scalar(
                out=thr_c[:, :], in0=clipl_c[:, :],
                scalar1=0.5, scalar2=-0.75, op0=Alu.mult, op1=Alu.add)
            clm1_c = cp.tile([NS, 1], f32, tag="clm1_c")
            nc.vector.tensor_scalar(
                out=clm1_c[:, :], in0=clipl_c[:, :],
                scalar1=-1.0, scalar2=None, op0=Alu.add)

            # row (partition-0) scalars for PE broadcast
            lenr_f = cp.tile([1, NS], f32, tag="lenr_f")
            nc.vector.tensor_copy(out=lenr_f[:, :], in_=lenr_i[:, :])
            clipl_r = cp.tile([1, NS], f32, tag="clipl_r")
            nc.vector.tensor_scalar(
                out=clipl_r[:, :], in0=lenr_f[:, :],
                scalar1=16.0, scalar2=None, op0=Alu.max)
            thr_r = cp.tile([1, NS], f32, tag="thr_r")
            nc.vector.tensor_scalar(
                out=thr_r[:, :], in0=clipl_r[:, :],
                scalar1=0.5, scalar2=-0.75, op0=Alu.mult, op1=Alu.add)
            clm1_r = cp.tile([1, NS], f32, tag="clm1_r")
            nc.vector.tensor_scalar(
                out=clm1_r[:, :], in0=clipl_r[:, :],
                scalar1=-1.0, scalar2=None, op0=Alu.add)

            # veff rows [NS, L]
            cand_r = cp.tile([NS, L], f32, tag="cand_r")
            nc.vector.tensor_tensor(
                out=cand_r[:, :], in0=iotar_t[:, :],
                in1=clm1_c[:, :].to_broadcast([NS, L]), op=Alu.is_lt)
            veff = cp.tile([NS, L], f32, tag="veff")
            nc.vector.tensor_scalar(
                out=veff[:, :], in0=noise_t[:, :],
                scalar1=-9.0, scalar2=None, op0=Alu.add)
            nc.vector.tensor_tensor(
                out=veff[:, :], in0=veff[:, :], in1=cand_r[:, :], op=Alu.mult)
            nc.vector.tensor_scalar(
                out=veff[:, :], in0=veff[:, :],
                scalar1=9.0, scalar2=None, op0=Alu.add)

            # attention mask rows + invert (independent of noise)
            attn_rows = cp.tile([NS, L], f32, tag="attn_rows")
            nc.vector.tensor_tensor(
                out=attn_rows[:, :], in0=iotar_t[:, :],
                in1=thr_c[:, :].to_broadcast([NS, L]), op=Alu.is_lt)
            nc.sync.dma_start(out=om_d[:, :], in_=attn_rows[:, :])
            inv_rows = cp.tile([NS, L], f32, tag="inv_rows")
            nc.vector.tensor_scalar(
                out=inv_rows[:, :], in0=attn_rows[:, :],
                scalar1=-1.0, scalar2=1.0, op0=Alu.mult, op1=Alu.add)
            nc.sync.dma_start(out=omi_d[:, :], in_=inv_rows[:, :])

            thrS_r = cp.tile([1, NS], f32, tag="thrS_r")
            nc.vector.tensor_scalar(
                out=thrS_r[:, :], in0=thr_r[:, :],
                scalar1=2.0, scalar2=-511.0, op0=Alu.mult, op1=Alu.add)

            # broadcast per-sample scalars to [P, NS] via K=1 outer products
            bc_ps = pss.tile([P, 4 * NS], f32, tag="smallmm")
            nc.tensor.matmul(out=bc_ps[:, 0:NS], lhsT=ones1[:, :],
                             rhs=thr_r[:, :], start=True, stop=True)
            nc.tensor.matmul(out=bc_ps[:, NS:2 * NS], lhsT=ones1[:, :],
                             rhs=clipl_r[:, :], start=True, stop=True)
            nc.tensor.matmul(out=bc_ps[:, 2 * NS:3 * NS], lhsT=ones1[:, :],
                             rhs=clm1_r[:, :], start=True, stop=True)
            nc.tensor.matmul(out=bc_ps[:, 3 * NS:4 * NS], lhsT=ones1[:, :],
                             rhs=thrS_r[:, :], start=True, stop=True)
            bc_sb = cp.tile([P, 4 * NS], f32, tag="bc_sb")
            nc.vector.tensor_copy(out=bc_sb[:, :], in_=bc_ps[:, :])
            thr_b = bc_sb[:, 0:NS]
            clipl_b = bc_sb[:, NS:2 * NS]
            clm1_b = bc_sb[:, 2 * NS:3 * NS]
            thrS_b = bc_sb[:, 3 * NS:4 * NS]

            # veff rows -> DRAM scratch (for partition-broadcast reads)
            nc.sync.dma_start(out=vscr_d[:, :], in_=veff[:, :])

            # veff as columns: transpose each [NS,128] slice -> [128,NS]
            vcolT = cp.tile([P, NCH * NS], f32, tag="vcolT")
            for c in range(NCH):
                vtr = pss.tile([P, NS], f32, tag="smallmm")
                nc.tensor.transpose(
                    out=vtr[:, :], in_=veff[:, c * P:(c + 1) * P],
                    identity=ident_t[:NS, :NS])
                nc.vector.tensor_copy(
                    out=vcolT[:, c * NS:(c + 1) * NS], in_=vtr[:, :])

            keep_all = cp.tile([P, NCH * NS], f32, tag="keep_all")

            # prefetch all replicated veff rows (depend only on setup)
            v_reps = []
            for n in range(NS):
                v_rep = wp.tile([P, L], f32, tag=f"v_rep{n}")
                nc.sync.dma_start(
                    out=v_rep[:, :],
                    in_=vscr_d[n:n + 1, :].to_broadcast([P, L]))
                v_reps.append(v_rep)

            # ---------- per-sample / per-group main pipeline ----------
            for g in range(NGRP):
                keep_g = wp.tile([P, NCH * G], bf16, tag="keep_g")
                for s in range(G):
                    n = g * G + s
                    v_rep = v_reps[n]
                    # rank[p] = #{j: veff[j] < veff_col[p]}
                    # chunks 0-1 on DVE (is_lt count), 2-3 on ACT (Sign sum)
                    # Non-candidates (veff=9.0) always rank >= len_keep, so
                    # keep = (rank < len_keep) needs no candidate mask.
                    rank2 = wp.tile([P, 2], f32, tag="rank2")
                    rankS = wp.tile([P, NCH - 2], f32, tag="rankS")
                    for c in range(2):
                        scr = wp.tile([P, L], f32, tag="scr")
                        nc.vector.tensor_scalar(
                            out=scr[:, :], in0=v_rep[:, :],
                            scalar1=vcolT[:, c * NS + n:c * NS + n + 1],
                            scalar2=None, op0=Alu.is_lt, op1=Alu.add,
                            accum_out=rank2[:, c:c + 1])
                    for c in range(2, NCH):
                        scra = wp.tile([P, L], f32, tag="scra")
                        nc.scalar.activation(
                            out=scra[:, :], in_=v_rep[:, :], func=Act.Sign,
                            scale=-1.0,
                            bias=vcolT[:, c * NS + n:c * NS + n + 1],
                            accum_out=rankS[:, c - 2:c - 1])
                    nc.vector.tensor_tensor(
                        out=keep_all[:, n * NCH:n * NCH + 2], in0=rank2[:, :],
                        in1=thr_b[:, n:n + 1].to_broadcast([P, 2]),
                        op=Alu.is_lt)
                    # Sign-sum S: keep <=> S < 2*len_keep-511 (thrS)
                    nc.vector.tensor_tensor(
                        out=keep_all[:, n * NCH + 2:(n + 1) * NCH],
                        in0=rankS[:, :],
                        in1=thrS_b[:, n:n + 1].to_broadcast([P, NCH - 2]),
                        op=Alu.is_lt)
                    # chunk-major bf16 copy: col k*G+s (exact 0/1 values)
                    nc.vector.tensor_copy(
                        out=keep_g[:, s::G],
                        in_=keep_all[:, n * NCH:(n + 1) * NCH])

                # inclusive prefix sums over positions (both samples at once)
                c_ps = psb.tile([P, NCH * G], f32, tag="cps")
                for m in range(NCH):
                    for k in range(m + 1):
                        nc.tensor.matmul(
                            out=c_ps[:, m * G:(m + 1) * G],
                            lhsT=(triu_bf[:, :] if k == m
                                  else ones128_bf[:, :]),
                            rhs=keep_g[:, k * G:(k + 1) * G],
                            start=(k == 0), stop=(k == m))

                # invert the compaction: src_j = #{i: c_i <= j}
                # mic[p, f] = (c[ic*128+p] <= jrow[f]) with jrow in pair
                # order (col h*128+q -> j = 2q+h). Column sums via PE with
                # mic slices as lhsT: out[q] = src_{2q+h} directly.
                c_sb = wp.tile([P, NCH * G], f32, tag="c_sb")
                nc.vector.tensor_copy(out=c_sb[:, :], in_=c_ps[:, :])
                for s in range(G):
                    n = g * G + s
                    mics = []
                    for ic in range(NCH):
                        mic = wp.tile([P, JCH * P], bf16, tag="mic")
                        nc.vector.tensor_tensor(
                            out=mic[:, :],
                            in0=c_sb[:, ic * G + s:ic * G + s + 1]
                                .to_broadcast([P, JCH * P]),
                            in1=jrow_t[:, :], op=Alu.is_le)
                        mics.append(mic)
                    src_ps = pss.tile([P, JCH], f32, tag="srccol")
                    for h in range(JCH):
                        for ic in range(NCH):
                            nc.tensor.matmul(
                                out=src_ps[:, h:h + 1],
                                lhsT=mics[ic][:, h * P:(h + 1) * P],
                                rhs=ones128_bf[:, 0:1],
                                start=(ic == 0), stop=(ic == NCH - 1))
                    # o0 = 1e6 for invalid pairs (src_{2p} == 512)
                    o0 = wp.tile([P, 1], f32, tag="o0")
                    nc.vector.tensor_scalar(
                        out=o0[:, :], in0=src_ps[:, 0:1],
                        scalar1=511.5, scalar2=1.0e6,
                        op0=Alu.is_ge, op1=Alu.mult)
                    # invalid pairs -> OOB on both gather halves (skipped);
                    # valid pairs keep idx 512 (zero row) at the boundary
                    srcadj = wp.tile([P, JCH], f32, tag="srcadj")
                    nc.vector.tensor_tensor(
                        out=srcadj[:, :], in0=src_ps[:, :],
                        in1=o0[:, :].to_broadcast([P, JCH]), op=Alu.add)
                    srci = wp.tile([P, JCH], i32, tag="srci")
                    nc.vector.tensor_copy(out=srci[:, :], in_=srcadj[:, :])
                    dest2 = wp.tile([P, 1], f32, tag="dest2")
                    nc.vector.tensor_scalar(
                        out=dest2[:, :], in0=o0[:, :],
                        scalar1=float(n * L), scalar2=None, op0=Alu.add)
                    nc.vector.tensor_tensor(
                        out=dest2[:, :], in0=dest2[:, :], in1=iotae_t[:, :],
                        op=Alu.add)
                    desti = wp.tile([P, 1], i32, tag="desti")
                    nc.vector.tensor_copy(out=desti[:, :], in_=dest2[:, :])

                    # gather rows 2p+h into halves of a pair tile; rows at
                    # j >= len_keep read the per-sample zero row (idx 512)
                    gtile = gp.tile([P, 2 * D], f32, tag="gtile")
                    for h in range(JCH):
                        nc.gpsimd.indirect_dma_start(
                            out=gtile[:, h * D:(h + 1) * D],
                            out_offset=None,
                            in_=x_d[:, :],
                            in_offset=bass.IndirectOffsetOnAxis(
                                ap=srci[:, h:h + 1], axis=0),
                            element_offset=n * (L + 1) * D,
                            bounds_check=L,
                            oob_is_err=False,
                        )
                    # one scatter writes row pairs (2 consecutive out rows
                    # per offset); pairs past len_keep are skipped via OOB
                    nc.gpsimd.indirect_dma_start(
                        out=ox_d[:, :],
                        out_offset=bass.IndirectOffsetOnAxis(
                            ap=desti[:, :], axis=0),
                        in_=gtile[:, :],
                        in_offset=None,
                        bounds_check=NS * L - 1,
                        oob_is_err=False,
                    )

            # ---------- removed mask (off the critical path) ----------
            for n in range(NS):
                inclc = wp.tile([P, NCH], f32, tag="inclc")
                nc.vector.tensor_tensor(
                    out=inclc[:, :], in0=iota4_t[:, :],
                    in1=clipl_b[:, n:n + 1].to_broadcast([P, NCH]),
                    op=Alu.is_lt)
                remc = wp.tile([P, NCH], f32, tag="remc")
                nc.vector.tensor_tensor(
                    out=remc[:, :], in0=inclc[:, :],
                    in1=keep_all[:, n * NCH:(n + 1) * NCH],
                    op=Alu.subtract)
                # transpose to [NCH, 128]: store = 4 contiguous 512B runs
                rem_ps = pss.tile([NCH, P], f32, tag="smallmm")
                nc.tensor.transpose(
                    out=rem_ps[:, :], in_=remc[:, :],
                    identity=ident_t[:, :])
                rem_sb = wp.tile([NCH, P], f32, tag="rem_sb")
                nc.vector.tensor_copy(out=rem_sb[:, :], in_=rem_ps[:, :])
                nc.sync.dma_start(
                    out=orm_d[n:n + 1, :].rearrange(
                        "o (c p) -> (o c) p", p=P),
                    in_=rem_sb[:, :])

    nc.finalize()
    return nc


def _consts():
    p = np.arange(P, dtype=np.float32)
    iota4 = np.stack([p + c * P for c in range(NCH)], axis=1)
    # pair order: col h*128+q corresponds to output row j = 2q+h
    jr = np.zeros(JCH * P, np.float32)
    for h in range(JCH):
        jr[h * P:(h + 1) * P] = 2 * np.arange(P) + h
    jrow = np.tile(jr[None, :], (P, 1))
    iotar = np.tile(np.arange(L, dtype=np.float32)[None, :], (NS, 1))
    cmb = np.concatenate([
        np.eye(P, dtype=np.float32),
        np.triu(np.ones((P, P), np.float32)),
        iota4,
        2.0 * p[:, None],
        jrow,
    ], axis=1)
    return {
        "cmb": np.ascontiguousarray(cmb),
        "iotar": np.ascontiguousarray(iotar),
    }


def _get_nc():
    if "nc" not in _nc_cache:
        _nc_cache["nc"] = build_nc()
    return _nc_cache["nc"]


def _pad_x(xs):
    xp = np.zeros((NS, L + 1, D), np.float32)
    xp[:, :L] = xs
    return xp.reshape(NS * (L + 1), D)


def make_in_maps(x, noise, lengths):
    consts = _consts()
    in_maps = []
    for ci in range(NCORES):
        sl = slice(ci * NS, (ci + 1) * NS)
        in_maps.append({
            "x": _pad_x(x[sl]),
            "noise": np.ascontiguousarray(noise[sl], np.float32),
            "lengths": np.ascontiguousarray(lengths[sl], np.int32),
            **consts,
        })
    return in_maps


def assemble(results):
    mx = np.concatenate(
        [r["out_x"].reshape(NS, L, D) for r in results], axis=0)
    m = np.concatenate([r["out_m"] for r in results], axis=0)
    mi = np.concatenate([r["out_mi"] for r in results], axis=0)
    rm = np.concatenate([r["out_r"] for r in results], axis=0)
    return mx, m, mi, rm


def kernel(x, noise, lengths, trace=False):
    from concourse.bass_utils import run_bass_kernel_spmd

    nc = _get_nc()
    in_maps = make_in_maps(x, noise, lengths)
    res = run_bass_kernel_spmd(nc, in_maps, list(range(NCORES)), trace=trace)
    out = assemble(res.results)
    if trace:
        return out, res
    return out


# revision 24
# speedup vs baseline: 1.0704x; 1.0704x over previous
"""Trainium2 Bass kernel for batched EEG masking-preserve-order (ragged gather).

Contract: kernel(x, noise, lengths) takes FULL inputs (N=64, L=512, D=840),
shards the batch over 8 NeuronCores (8 samples each), runs a Bass/Tile
kernel per core, and reassembles full-shape outputs:
  (masked_x [N,L,D], masked_attention_mask [N,L],
   masked_attention_mask_invert [N,L], removed_mask [N,L])

Per-sample algorithm (validated bit-exact vs the jax reference):
  clipl  = max(lengths, 16);  len_keep = floor(clipl/2)
  cand_i = i < clipl-1
  veff_i = cand_i ? noise_i : 9.0
  rank_i = #{j : veff_j < veff_i}        (no ties among candidates)
  keep_i = cand_i & (rank_i < len_keep)
  c_i    = inclusive prefix sum of keep  (PE matmuls w/ triangular ones)
  src_j  = #{i : c_i <= j}               (position of (j+1)-th kept row;
                                          = 512 OOB when j >= len_keep)
  out[j] = x[src_j] for j < len_keep, else 0.
Heavy data movement: per (sample, 128-row output chunk) one indirect-DMA
gather with [128,1] row offsets + one indirect-DMA scatter-store (both skip
masked rows via bounds_check OOB). Unwritten output rows rely on pre-zeroed
ExternalOutput buffers. veff row replication uses a DRAM-scratch broadcast
read; rank counting is split between the Vector (is_lt count) and Scalar
(Sign-sum) engines; PE matmuls use bf16 0/1 operands (exact, full-rate).
"""

import numpy as np

N, L, D = 64, 512, 840
NCORES = 8
NS = N // NCORES        # samples per core
P = 128
NCH = L // P            # position chunks per sample
JCH = 2                 # output-row chunks per sample (len_keep <= 256)
G = 2                   # samples per prefix-matmul batch
NGRP = NS // G

_nc_cache = {}


def build_nc():
    import concourse.bass as bass
    import concourse.bacc as bacc
    import concourse.mybir as mybir
    from concourse.tile import TileContext

    f32 = mybir.dt.float32
    bf16 = mybir.dt.bfloat16
    i32 = mybir.dt.int32
    Alu = mybir.AluOpType
    Act = mybir.ActivationFunctionType

    nc = bacc.Bacc()

    x_d = nc.dram_tensor("x", [NS * (L + 1), D], f32, kind="ExternalInput")
    noise_d = nc.dram_tensor("noise", [NS, L], f32, kind="ExternalInput")
    len_d = nc.dram_tensor("lengths", [NS], i32, kind="ExternalInput")
    CMB = 2 * P + NCH + 1 + JCH * P
    cmb_d = nc.dram_tensor("cmb", [P, CMB], f32, kind="ExternalInput")
    iotar_d = nc.dram_tensor("iotar", [NS, L], f32, kind="ExternalInput")

    ox_d = nc.dram_tensor("out_x", [NS * L, D], f32, kind="ExternalOutput")
    om_d = nc.dram_tensor("out_m", [NS, L], f32, kind="ExternalOutput")
    omi_d = nc.dram_tensor("out_mi", [NS, L], f32, kind="ExternalOutput")
    orm_d = nc.dram_tensor("out_r", [NS, L], f32, kind="ExternalOutput")
    vscr_d = nc.dram_tensor("veff_scr", [NS, L], f32)

    with TileContext(nc) as tc:
        with (
            tc.tile_pool(name="const", bufs=1) as cp,
            tc.tile_pool(name="work", bufs=4) as wp,
            tc.tile_pool(name="gp", bufs=4) as gp,
            tc.tile_pool(name="psb", bufs=2, space="PSUM") as psb,
            tc.tile_pool(name="pss", bufs=2, space="PSUM") as pss,
        ):
            # ---------- setup ----------
            # data-path loads first (they gate the whole pipeline)
            noise_t = cp.tile([NS, L], f32, tag="noise")
            nc.sync.dma_start(out=noise_t[:, :], in_=noise_d[:, :])
            lenc_i = cp.tile([NS, 1], i32, tag="lenc_i")
            nc.sync.dma_start(out=lenc_i[:, :], in_=len_d[:, None])
            lenr_i = cp.tile([1, NS], i32, tag="lenr_i")
            nc.sync.dma_start(out=lenr_i[:, :], in_=len_d[None, :])
            iotar_t = cp.tile([NS, L], f32, tag="iotar")
            nc.sync.dma_start(out=iotar_t[:, :], in_=iotar_d[:, :])
            # all [128,*] constants in one combined load
            cmb_t = cp.tile([P, CMB], f32, tag="cmb")
            nc.sync.dma_start(out=cmb_t[:, :], in_=cmb_d[:, :])
            ident_t = cmb_t[:, 0:P]
            triu_t = cmb_t[:, P:2 * P]
            iota4_t = cmb_t[:, 2 * P:2 * P + NCH]
            iotae_t = cmb_t[:, 2 * P + NCH:2 * P + NCH + 1]
            jrow_t = cmb_t[:, 2 * P + NCH + 1:CMB]

            ones1 = cp.tile([1, P], f32, tag="ones1")
            nc.vector.memset(ones1[:, :], 1.0)
            ones128 = cp.tile([P, P], f32, tag="ones128")
            nc.vector.memset(ones128[:, :], 1.0)
            triu_bf = cp.tile([P, P], bf16, tag="triu_bf")
            nc.vector.tensor_copy(out=triu_bf[:, :], in_=triu_t[:, :])
            ones128_bf = cp.tile([P, P], bf16, tag="ones128_bf")
            nc.vector.tensor_copy(out=ones128_bf[:, :], in_=ones128[:, :])

            # column (per-sample-on-partition) scalars
            lenc_f = cp.tile([NS, 1], f32, tag="lenc_f")
            nc.vector.tensor_copy(out=lenc_f[:, :], in_=lenc_i[:, :])
            clipl_c = cp.tile([NS, 1], f32, tag="clipl_c")
            nc.vector.tensor_scalar(
                out=clipl_c[:, :], in0=lenc_f[:, :],
                scalar1=16.0, scalar2=None, op0=Alu.max)
            thr_c = cp.tile([NS, 1], f32, tag="thr_c")
            nc.vector.tensor_scalar(
                out=thr_c[:, :], in0=clipl_c[:, :],
                scalar1=0.5, scalar2=-0.75, op0=Alu.mult, op1=Alu.add)
            clm1_c = cp.tile([NS, 1], f32, tag="clm1_c")
            nc.vector.tensor_scalar(
                out=clm1_c[:, :], in0=clipl_c[:, :],
                scalar1=-1.0, scalar2=None, op0=Alu.add)

            # row (partition-0) scalars for PE broadcast
            lenr_f = cp.tile([1, NS], f32, tag="lenr_f")
            nc.vector.tensor_copy(out=lenr_f[:, :], in_=lenr_i[:, :])
            clipl_r = cp.tile([1, NS], f32, tag="clipl_r")
            nc.vector.tensor_scalar(
                out=clipl_r[:, :], in0=lenr_f[:, :],
                scalar1=16.0, scalar2=None, op0=Alu.max)
            thr_r = cp.tile([1, NS], f32, tag="thr_r")
            nc.vector.tensor_scalar(
                out=thr_r[:, :], in0=clipl_r[:, :],
                scalar1=0.5, scalar2=-0.75, op0=Alu.mult, op1=Alu.add)
            clm1_r = cp.tile([1, NS], f32, tag="clm1_r")
            nc.vector.tensor_scalar(
                out=clm1_r[:, :], in0=clipl_r[:, :],
                scalar1=-1.0, scalar2=None, op0=Alu.add)

            # veff rows [NS, L]
            cand_r = cp.tile([NS, L], f32, tag="cand_r")
            nc.vector.tensor_tensor(
                out=cand_r[:, :], in0=iotar_t[:, :],
                in1=clm1_c[:, :].to_broadcast([NS, L]), op=Alu.is_lt)
            veff = cp.tile([NS, L], f32, tag="veff")
            nc.vector.tensor_scalar(
                out=veff[:, :], in0=noise_t[:, :],
                scalar1=-9.0, scalar2=None, op0=Alu.add)
            nc.vector.tensor_tensor(
                out=veff[:, :], in0=veff[:, :], in1=cand_r[:, :], op=Alu.mult)
            nc.vector.tensor_scalar(
                out=veff[:, :], in0=veff[:, :],
                scalar1=9.0, scalar2=None, op0=Alu.add)

            thrS_r = cp.tile([1, NS], f32, tag="thrS_r")
            nc.vector.tensor_scalar(
                out=thrS_r[:, :], in0=thr_r[:, :],
                scalar1=2.0, scalar2=-511.0, op0=Alu.mult, op1=Alu.add)

            # broadcast per-sample scalars to [P, NS] via K=1 outer products
            bc_ps = pss.tile([P, 4 * NS], f32, tag="smallmm")
            nc.tensor.matmul(out=bc_ps[:, 0:NS], lhsT=ones1[:, :],
                             rhs=thr_r[:, :], start=True, stop=True)
            nc.tensor.matmul(out=bc_ps[:, NS:2 * NS], lhsT=ones1[:, :],
                             rhs=clipl_r[:, :], start=True, stop=True)
            nc.tensor.matmul(out=bc_ps[:, 2 * NS:3 * NS], lhsT=ones1[:, :],
                             rhs=clm1_r[:, :], start=True, stop=True)
            nc.tensor.matmul(out=bc_ps[:, 3 * NS:4 * NS], lhsT=ones1[:, :],
                             rhs=thrS_r[:, :], start=True, stop=True)
            bc_sb = cp.tile([P, 4 * NS], f32, tag="bc_sb")
            nc.vector.tensor_copy(out=bc_sb[:, :], in_=bc_ps[:, :])
            thr_b = bc_sb[:, 0:NS]
            clipl_b = bc_sb[:, NS:2 * NS]
            clm1_b = bc_sb[:, 2 * NS:3 * NS]
            thrS_b = bc_sb[:, 3 * NS:4 * NS]

            # veff rows -> DRAM scratch (for partition-broadcast reads)
            nc.sync.dma_start(out=vscr_d[:, :], in_=veff[:, :])

            # veff as columns: transpose each [NS,128] slice -> [128,NS]
            vcolT = cp.tile([P, NCH * NS], f32, tag="vcolT")
            for c in range(NCH):
                vtr = pss.tile([P, NS], f32, tag="smallmm")
                nc.tensor.transpose(
                    out=vtr[:, :], in_=veff[:, c * P:(c + 1) * P],
                    identity=ident_t[:NS, :NS])
                nc.vector.tensor_copy(
                    out=vcolT[:, c * NS:(c + 1) * NS], in_=vtr[:, :])

            keep_all = cp.tile([P, NCH * NS], f32, tag="keep_all")

            # prefetch all replicated veff rows (depend only on setup)
            v_reps = []
            for n in range(NS):
                v_rep = wp.tile([P, L], f32, tag=f"v_rep{n}")
                nc.sync.dma_start(
                    out=v_rep[:, :],
                    in_=vscr_d[n:n + 1, :].to_broadcast([P, L]))
                v_reps.append(v_rep)

            # ---------- fully per-sample pipeline (G=1) ----------
            for n in range(NS):
                v_rep = v_reps[n]
                # rank[p] = #{j: veff[j] < veff_col[p]}
                # chunks 0-1 on DVE (is_lt count), 2-3 on ACT (Sign sum).
                # Non-candidates (veff=9.0) always rank >= len_keep, so
                # keep = (rank < len_keep) needs no candidate mask.
                rank2 = wp.tile([P, 2], f32, tag="rank2")
                rankS = wp.tile([P, NCH - 2], f32, tag="rankS")
                for c in range(2):
                    scr = wp.tile([P, L], f32, tag="scr")
                    nc.vector.tensor_scalar(
                        out=scr[:, :], in0=v_rep[:, :],
                        scalar1=vcolT[:, c * NS + n:c * NS + n + 1],
                        scalar2=None, op0=Alu.is_lt, op1=Alu.add,
                        accum_out=rank2[:, c:c + 1])
                for c in range(2, NCH):
                    scra = wp.tile([P, L], f32, tag="scra")
                    nc.scalar.activation(
                        out=scra[:, :], in_=v_rep[:, :], func=Act.Sign,
                        scale=-1.0,
                        bias=vcolT[:, c * NS + n:c * NS + n + 1],
                        accum_out=rankS[:, c - 2:c - 1])
                nc.vector.tensor_tensor(
                    out=keep_all[:, n * NCH:n * NCH + 2], in0=rank2[:, :],
                    in1=thr_b[:, n:n + 1].to_broadcast([P, 2]),
                    op=Alu.is_lt)
                # Sign-sum S: keep <=> S < 2*len_keep-511 (thrS)
                nc.vector.tensor_tensor(
                    out=keep_all[:, n * NCH + 2:(n + 1) * NCH],
                    in0=rankS[:, :],
                    in1=thrS_b[:, n:n + 1].to_broadcast([P, NCH - 2]),
                    op=Alu.is_lt)
                keep_bf = wp.tile([P, NCH], bf16, tag="keep_bf")
                nc.vector.tensor_copy(
                    out=keep_bf[:, :], in_=keep_all[:, n * NCH:(n + 1) * NCH])

                # inclusive prefix sum over positions (PE, bf16 0/1 exact)
                c_ps = psb.tile([P, NCH], f32, tag="cps")
                for m in range(NCH):
                    for k in range(m + 1):
                        nc.tensor.matmul(
                            out=c_ps[:, m:m + 1],
                            lhsT=(triu_bf[:, :] if k == m
                                  else ones128_bf[:, :]),
                            rhs=keep_bf[:, k:k + 1],
                            start=(k == 0), stop=(k == m))
                c_sb = wp.tile([P, NCH], f32, tag="c_sb")
                nc.vector.tensor_copy(out=c_sb[:, :], in_=c_ps[:, :])

                # invert the compaction: src_j = #{i: c_i <= j}
                # mic[p, f] = (c[ic*128+p] <= jrow[f]), jrow in pair order
                # (col h*128+q -> j = 2q+h); PE column sums with mic as lhsT
                mics = []
                for ic in range(NCH):
                    mic = wp.tile([P, JCH * P], bf16, tag="mic")
                    nc.vector.tensor_tensor(
                        out=mic[:, :],
                        in0=c_sb[:, ic:ic + 1].to_broadcast([P, JCH * P]),
                        in1=jrow_t[:, :], op=Alu.is_le)
                    mics.append(mic)
                src_ps = pss.tile([P, JCH], f32, tag="srccol")
                for h in range(JCH):
                    for ic in range(NCH):
                        nc.tensor.matmul(
                            out=src_ps[:, h:h + 1],
                            lhsT=mics[ic][:, h * P:(h + 1) * P],
                            rhs=ones128_bf[:, 0:1],
                            start=(ic == 0), stop=(ic == NCH - 1))
                # o0 = 1e6 for invalid pairs (src_{2p} == 512)
                o0 = wp.tile([P, 1], f32, tag="o0")
                nc.vector.tensor_scalar(
                    out=o0[:, :], in0=src_ps[:, 0:1],
                    scalar1=511.5, scalar2=1.0e6,
                    op0=Alu.is_ge, op1=Alu.mult)
                # invalid pairs -> OOB on both gather halves (skipped);
                # valid pairs keep idx 512 (zero row) at the boundary
                srcadj = wp.tile([P, JCH], f32, tag="srcadj")
                nc.vector.tensor_tensor(
                    out=srcadj[:, :], in0=src_ps[:, :],
                    in1=o0[:, :].to_broadcast([P, JCH]), op=Alu.add)
                srci = wp.tile([P, JCH], i32, tag="srci")
                nc.vector.tensor_copy(out=srci[:, :], in_=srcadj[:, :])
                dest2 = wp.tile([P, 1], f32, tag="dest2")
                nc.vector.tensor_scalar(
                    out=dest2[:, :], in0=o0[:, :],
                    scalar1=float(n * L), scalar2=None, op0=Alu.add)
                nc.vector.tensor_tensor(
                    out=dest2[:, :], in0=dest2[:, :], in1=iotae_t[:, :],
                    op=Alu.add)
                desti = wp.tile([P, 1], i32, tag="desti")
                nc.vector.tensor_copy(out=desti[:, :], in_=dest2[:, :])

                # gather rows 2p+h into halves of a pair tile; the boundary
                # row of a valid pair reads the per-sample zero row (idx 512)
                gtile = gp.tile([P, 2 * D], f32, tag="gtile")
                for h in range(JCH):
                    nc.gpsimd.indirect_dma_start(
                        out=gtile[:, h * D:(h + 1) * D],
                        out_offset=None,
                        in_=x_d[:, :],
                        in_offset=bass.IndirectOffsetOnAxis(
                            ap=srci[:, h:h + 1], axis=0),
                        element_offset=n * (L + 1) * D,
                        bounds_check=L,
                        oob_is_err=False,
                    )
                # one scatter writes row pairs (2 consecutive out rows per
                # offset); pairs past len_keep are skipped via OOB
                nc.gpsimd.indirect_dma_start(
                    out=ox_d[:, :],
                    out_offset=bass.IndirectOffsetOnAxis(
                        ap=desti[:, :], axis=0),
                    in_=gtile[:, :],
                    in_offset=None,
                    bounds_check=NS * L - 1,
                    oob_is_err=False,
                )

            # ---------- attention masks (off the critical path) ----------
            attn_rows = cp.tile([NS, L], f32, tag="attn_rows")
            nc.vector.tensor_tensor(
                out=attn_rows[:, :], in0=iotar_t[:, :],
                in1=thr_c[:, :].to_broadcast([NS, L]), op=Alu.is_lt)
            nc.sync.dma_start(out=om_d[:, :], in_=attn_rows[:, :])
            inv_rows = cp.tile([NS, L], f32, tag="inv_rows")
            nc.vector.tensor_scalar(
                out=inv_rows[:, :], in0=attn_rows[:, :],
                scalar1=-1.0, scalar2=1.0, op0=Alu.mult, op1=Alu.add)
            nc.sync.dma_start(out=omi_d[:, :], in_=inv_rows[:, :])

            # ---------- removed mask (off the critical path) ----------
            for n in range(NS):
                inclc = wp.tile([P, NCH], f32, tag="inclc")
                nc.vector.tensor_tensor(
                    out=inclc[:, :], in0=iota4_t[:, :],
                    in1=clipl_b[:, n:n + 1].to_broadcast([P, NCH]),
                    op=Alu.is_lt)
                remc = wp.tile([P, NCH], f32, tag="remc")
                nc.vector.tensor_tensor(
                    out=remc[:, :], in0=inclc[:, :],
                    in1=keep_all[:, n * NCH:(n + 1) * NCH],
                    op=Alu.subtract)
                # transpose to [NCH, 128]: store = 4 contiguous 512B runs
                rem_ps = pss.tile([NCH, P], f32, tag="smallmm")
                nc.tensor.transpose(
                    out=rem_ps[:, :], in_=remc[:, :],
                    identity=ident_t[:, :])
                rem_sb = wp.tile([NCH, P], f32, tag="rem_sb")
                nc.vector.tensor_copy(out=rem_sb[:, :], in_=rem_ps[:, :])
                nc.sync.dma_start(
                    out=orm_d[n:n + 1, :].rearrange(
                        "o (c p) -> (o c) p", p=P),
                    in_=rem_sb[:, :])

    nc.finalize()
    return nc


def _consts():
    p = np.arange(P, dtype=np.float32)
    iota4 = np.stack([p + c * P for c in range(NCH)], axis=1)
    # pair order: col h*128+q corresponds to output row j = 2q+h
    jr = np.zeros(JCH * P, np.float32)
    for h in range(JCH):
        jr[h * P:(h + 1) * P] = 2 * np.arange(P) + h
    jrow = np.tile(jr[None, :], (P, 1))
    iotar = np.tile(np.arange(L, dtype=np.float32)[None, :], (NS, 1))
    cmb = np.concatenate([
        np.eye(P, dtype=np.float32),
        np.triu(np.ones((P, P), np.float32)),
        iota4,
        2.0 * p[:, None],
        jrow,
    ], axis=1)
    return {
        "cmb": np.ascontiguousarray(cmb),
        "iotar": np.ascontiguousarray(iotar),
    }


def _get_nc():
    if "nc" not in _nc_cache:
        _nc_cache["nc"] = build_nc()
    return _nc_cache["nc"]


def _pad_x(xs):
    xp = np.zeros((NS, L + 1, D), np.float32)
    xp[:, :L] = xs
    return xp.reshape(NS * (L + 1), D)


def make_in_maps(x, noise, lengths):
    consts = _consts()
    in_maps = []
    for ci in range(NCORES):
        sl = slice(ci * NS, (ci + 1) * NS)
        in_maps.append({
            "x": _pad_x(x[sl]),
            "noise": np.ascontiguousarray(noise[sl], np.float32),
            "lengths": np.ascontiguousarray(lengths[sl], np.int32),
            **consts,
        })
    return in_maps


def assemble(results):
    mx = np.concatenate(
        [r["out_x"].reshape(NS, L, D) for r in results], axis=0)
    m = np.concatenate([r["out_m"] for r in results], axis=0)
    mi = np.concatenate([r["out_mi"] for r in results], axis=0)
    rm = np.concatenate([r["out_r"] for r in results], axis=0)
    return mx, m, mi, rm


def kernel(x, noise, lengths, trace=False):
    from concourse.bass_utils import run_bass_kernel_spmd

    nc = _get_nc()
    in_maps = make_in_maps(x, noise, lengths)
    res = run_bass_kernel_spmd(nc, in_maps, list(range(NCORES)), trace=trace)
    out = assemble(res.results)
    if trace:
        return out, res
    return out
